# revision 28
# baseline (speedup 1.0000x reference)
"""Trainium2 Bass kernel for CropConv (stride-2 3x3 conv, B=32 CIN=COUT=256,
H=W=64 -> 32x32; the crop mask is provably all-ones so output == conv output).

Strategy: data-parallel over batch across 8 NeuronCores (4 images/core).
Host pads each image to 66x66 and splits it into 4 polyphase components
(row/col parity), so each conv tap's moving operand is a fully contiguous
window. Per core the conv is 18 accumulated matmuls per 512-position output
tile: 9 taps x 2 cin-128-chunks, contracting cin on the PE partition dim.
PSUM accumulates fp32.

Precision split: weights are fp16 (11-bit mantissa), x ships as fp8 e3m4
(4-bit mantissa) — the PE upconverts both operands to fp22, so mixed-dtype
matmuls are native. e3m4 x-quantization alone gives 1.33e-2 max-rel error
on the reference inputs (gate 2e-2, deterministic seed); halving x DMA
bytes relaxes the HBM-delivery-bound startup ramp. Output is stored fp16
(|out| <= ~8, quantization ~5e-4 relative) and upcast on host.

Rejected faster paths (measured/derived this session):
- fp8 e4m3 DoubleRow (2x PE rate): needs BOTH operands e4m3; any product
  with an e4m3 factor costs >=2.9e-2 max-rel alone — over the gate. Hi/lo
  pair-splitting restores precision but doubles matmul count (= fp16).
- Winograd F(4,2)/F(2,2) on the polyphase components (1.4-1.8x fewer PE
  cols): the inverse transform needs ~5-12 elementwise ops/output on
  DVE/ACT at ~170ns+1ns/col per instruction — costs far more than the PE
  time saved. PSUM cannot be combined by the PE (write-only) or by DMA
  (no scaled/subtractive accumulate).
- N=1024 moving operands: matmul output is capped at one PSUM bank
  (512 fp32 cols); walrus ISA check s3d3_mm_num_elements rejects it.
- Strassen at the [256,2304]x[2304,1024] level: 7/8 matmuls but the 7
  B-side combos either re-quantize sums in fp8 (error ~2-3e-2) or ship
  1.75x the x bytes in fp16 (delivery-bound).

Timeline notes (from ntff traces): the graded exec window runs from the
runtime preamble (~6.1us untraced: engine rendezvous + TENSOR_LOAD) to
the end of the runtime's fixed ~8us semaphore-zeroing teardown; both are
NEFF-runtime costs, invariant to kernel structure. Matmul operands at
fp16/fp8 stream 1 col/cycle (216ns per 512-col matmul, gap-free). Eleven
warm-up matmuls on an UNINITIALIZED raw SBUF scratch (no memset
dependency; NaNs are constant-time in the fp22 PE pipeline and the
warm-up PSUM is never read) start PE activity at context entry, so the
HAM governor un-throttles the PE to 2.4GHz (~4.2us activity window)
about when the first real operands (w0 + image-0 chunks, ~2.0MB) finish
delivery. Fewer warm-ups measured WORSE: real matmuls hit delivery
stalls inside the HAM window and push un-throttle out by ~5us. The final
group is split half/quarter/quarter so earlier drains overlap the last
matmuls and the closing cast->HWDGE-DMA chain handles only 128 columns;
gpsimd (SWDGE) never appears near the end, keeping its dge-drain off the
critical path.
"""

import numpy as np
import ml_dtypes

import concourse.bacc as bacc
import concourse.mybir as mybir
import concourse.tile as tile
from concourse.bass_utils import run_bass_kernel_spmd

B, CIN, COUT, H, W = 32, 256, 256, 64, 64
OH, OW = 32, 32
NCORES = 8
BL = B // NCORES          # images per core
KC = CIN // 128           # cin chunks
MC = COUT // 128          # cout chunks
NT = 2                    # output row-halves per image (16 rows x 32 cols = 512)
RT = OH // NT             # out rows per tile
PR = 17                   # phase rows per half (16 + 1 halo)
PC = 33                   # phase cols
AW = 32                   # stored array width: 32-wide rows make every
                          # tap's rhs window a CONTIGUOUS 512 block
ASZ = PR * AW             # one (phase, col-shift) array = 544 elems
# (phase, c0) arrays stored per x half-tile. Only kx=2 taps read at col
# offset 1, so just phases 0 and 2 need the shifted duplicate.
COMBOS = [(0, 0), (1, 0), (2, 0), (3, 0), (0, 1), (2, 1)]
AIDX = {pc0: i for i, pc0 in enumerate(COMBOS)}
XHALF = len(COMBOS) * ASZ

# taps ordered c0=0 first then by phase — matches the flat array
# order the DMA pieces arrive in.
TAPS_C0 = sorted([(ky, kx) for ky in range(3) for kx in range(3)],
                 key=lambda t: (t[1] // 2, (t[0] % 2) * 2 + (t[1] % 2)))

_CACHE = {}


def _dtypes(mm_dtype):
    """(w_dtype, x_dtype) pairs by mode."""
    return {
        "f8": (mybir.dt.float16, mybir.dt.float8e3),
        "f16": (mybir.dt.float16, mybir.dt.float16),
        "bf16": (mybir.dt.bfloat16, mybir.dt.bfloat16),
    }[mm_dtype]


def _np_dtypes(mm_dtype):
    return {
        "f8": (np.float16, ml_dtypes.float8_e3m4),
        "f16": (np.float16, np.float16),
        "bf16": (ml_dtypes.bfloat16, ml_dtypes.bfloat16),
    }[mm_dtype]


def _build(mm_dtype="f8", nq=16):
    w_dt, x_dt = _dtypes(mm_dtype)
    nc = bacc.Bacc("TRN2", target_bir_lowering=False, debug=False,
                   num_devices=NCORES)
    if nq != 16:
        for q in nc.m.queues:
            q.num_queues = nq
    x = nc.dram_tensor("x", [BL, KC, NT, 128, XHALF], x_dt, kind="ExternalInput")
    w = nc.dram_tensor("w", [MC, 128, 9 * KC * 128], w_dt, kind="ExternalInput")
    # fp16 output: |out| <= ~8 so fp16 quantization adds only ~5e-4
    # relative error; halves output DMA bytes (host upcasts to fp32).
    y = nc.dram_tensor("y", [BL, MC, 128, OH * OW], mybir.dt.float16,
                       kind="ExternalOutput")

    n_warm = 11
    # Raw (non-pool) SBUF scratch for PE warm-up reads: bypasses the Tile
    # framework's written-before-read tracking so no memset is needed.
    warm_sb = nc.alloc_sbuf_tensor("warm_scratch", [128, 128 + RT * OW], w_dt)

    with tile.TileContext(nc) as tc:
        with (
            tc.tile_pool(name="wpool", bufs=MC) as wpool,
            tc.tile_pool(name="xpool", bufs=BL * KC * NT) as xpool,
            tc.tile_pool(name="opool", bufs=12) as opool,
            tc.tile_pool(name="spool", bufs=1) as spool,
            tc.tile_pool(name="psum", bufs=8, space="PSUM") as psum_pool,
        ):
            # PE warm-up: dummy matmuls keep the PE busy through the HAM
            # activity window while the first input DMAs are in flight, so
            # real matmuls start at 2.4 GHz. The scratch tile is read
            # UNINITIALIZED (garbage, possibly NaN): the fp22 PE pipeline
            # is constant-time on NaNs and the PSUM result is never read;
            # skipping the memset starts the HAM ramp ~0.3us earlier.
            if n_warm:
                wps = psum_pool.tile([128, RT * OW], mybir.dt.float32,
                                     name="warm_ps", tag="ps")
                for _ in range(n_warm):
                    nc.tensor.matmul(wps[:], warm_sb.ap()[:, :128],
                                     warm_sb.ap()[:, 128:], start=True,
                                     stop=True)

            # Input DMAs, finest first: the first matmul group gates only on
            # w[0] and image 0's first chunk, so those are split into small
            # pieces issued on independent trigger engines/queues.
            w_sb = {}
            x_sb = {}
            # gpsimd's SWDGE adds a ~3us dge-drain at kernel end, so it only
            # carries the first few critical pieces where a third parallel
            # queue shortens the startup; everything later uses the two
            # HWDGE queues (sync + scalar).
            trig3 = [nc.sync, nc.scalar, nc.gpsimd]
            trig2 = [nc.sync, nc.scalar]
            n_trig = 0

            def next_eng():
                nonlocal n_trig
                trig = trig3 if n_trig < 12 else trig2
                e = trig[n_trig % len(trig)]
                n_trig += 1
                return e

            w_sb[0] = wpool.tile([128, 9 * KC * 128], w_dt, tag="wsb", name="wsb0")
            w_sb[1] = wpool.tile([128, 9 * KC * 128], w_dt, tag="wsb", name="wsb1")

            # DMAs in PE-consumption order, small pieces round-robined over
            # the three trigger queues so each piece arrives just in time.
            for b in range(BL):
                for nt in range(NT):
                    for kc in range(KC):
                        x_sb[(b, kc, nt)] = xpool.tile(
                            [128, XHALF], x_dt,
                            tag="ximg", name=f"x_{b}_{kc}_{nt}")

            def w_piece(mc, i, n):
                step = 9 * KC * 128 // n
                sl = slice(i * step, (i + 1) * step)
                next_eng().dma_start(w_sb[mc][:, sl], w.ap()[mc][:, sl])

            def x_piece(b, kc, nt, i, n):
                step = XHALF // n
                sl = slice(i * step, (i + 1) * step)
                next_eng().dma_start(x_sb[(b, kc, nt)][:, sl],
                                     x.ap()[b, kc, nt, :, sl])

            w_piece(0, 0, 4)
            x_piece(0, 0, 0, 0, 4)
            x_piece(0, 0, 0, 1, 4)
            w_piece(0, 1, 4)
            x_piece(0, 0, 0, 2, 4)
            x_piece(0, 0, 0, 3, 4)
            x_piece(0, 1, 0, 0, 4)
            w_piece(0, 2, 4)
            x_piece(0, 1, 0, 1, 4)
            w_piece(0, 3, 4)
            x_piece(0, 1, 0, 2, 4)
            x_piece(0, 1, 0, 3, 4)
            for i in range(4):
                w_piece(1, i, 4)
            for kc in range(KC):
                for i in range(2):
                    x_piece(0, kc, 1, i, 2)
            for b in range(1, BL):
                for nt in range(NT):
                    for kc in range(KC):
                        for i in range(2):
                            x_piece(b, kc, nt, i, 2)

            out_trig = [nc.sync, nc.scalar]
            n_out = 0

            groups = [(mc, b, nt)
                      for b in range(BL)
                      for nt in range(NT)
                      for mc in range(MC)]
            def do_group(b, nt, mc, rh, n_rh, ps_cols, last_tile):
                """One accumulation group over output rows
                [rh*RT/n_rh, (rh+1)*RT/n_rh) of the (b, nt) half."""
                nonlocal n_out
                rt = RT // n_rh
                ps = psum_pool.tile([128, ps_cols], mybir.dt.float32,
                                    name=f"ps_{b}_{mc}_{nt}_{rh}", tag="ps")
                n_mm = KC * len(TAPS_C0)
                i_mm = 0
                for kc in range(KC):
                    xt = x_sb[(b, kc, nt)]
                    for (ky, kx) in TAPS_C0:
                        phase = (ky % 2) * 2 + (kx % 2)
                        r0 = ky // 2 + rh * rt
                        c0 = kx // 2
                        base = AIDX[(phase, c0)] * ASZ + r0 * AW
                        rhs = xt[:, base:base + rt * OW]
                        lhsT = w_sb[mc][:, (kc * 9 + ky * 3 + kx)
                                        * 128:][:, :128]
                        nc.tensor.matmul(
                            ps[:], lhsT, rhs,
                            start=(i_mm == 0), stop=(i_mm == n_mm - 1),
                        )
                        i_mm += 1
                base = nt * 512 + rh * rt * OW
                n_ch = 2 if n_rh == 1 else 1
                chunk = rt * OW // n_ch
                for h in range(n_ch):
                    ot = opool.tile([128, chunk], mybir.dt.float16,
                                    tag="ostage")
                    if last_tile:
                        # Tail latency: copy on an idle engine, DMA on the
                        # idle HWDGE queues (gpsimd SWDGE would add a
                        # dge-drain right at kernel end).
                        ceng = nc.vector if h == 0 else nc.scalar
                        deng = nc.sync if h == 0 else nc.scalar
                    else:
                        ceng = nc.vector
                        deng = out_trig[n_out % len(out_trig)]
                    n_out += 1  # round-robin across trigger engines
                    if ceng is nc.scalar:
                        ceng.copy(ot[:], ps[:, h * chunk:(h + 1) * chunk])
                    else:
                        ceng.tensor_copy(
                            ot[:], ps[:, h * chunk:(h + 1) * chunk])
                    deng.dma_start(
                        y.ap()[b, mc, :,
                               base + h * chunk:base + (h + 1) * chunk],
                        ot[:],
                    )

            for i_group, (mc, b, nt) in enumerate(groups):
                if i_group < len(groups) - 1:
                    do_group(b, nt, mc, 0, 1, RT * OW, False)
                else:
                    # Final group: a half then two quarter-height
                    # accumulations so earlier drains overlap the last
                    # matmuls and the closing cast->DMA chain handles
                    # only 128 columns.
                    do_group(b, nt, mc, 0, 2, RT * OW // 2, False)
                    do_group(b, nt, mc, 2, 4, RT * OW // 4, False)
                    do_group(b, nt, mc, 3, 4, RT * OW // 4, True)
    nc.compile()
    # Bass init unconditionally emits four const-AP memsets (fp32 0/1,
    # bf16 1, uint8 127) that nothing in this kernel reads — dead code
    # that also happens to run ~1.1us before the first DMA trigger.
    for blk in nc.m.functions[0].blocks:
        blk.instructions[:] = [
            i for i in blk.instructions
            if not ("Memset" in type(i).__name__ and "const-" in i.concise())
        ]
    return nc


def _get(mm_dtype="f8", nq=16):
    key = (mm_dtype, nq)
    if key not in _CACHE:
        _CACHE[key] = _build(mm_dtype, nq)
    return _CACHE[key]


def _prep_inputs(x, weight, mm_dtype="f8"):
    w_np, x_np = _np_dtypes(mm_dtype)
    # x: [B, CIN, H, W] -> pad to 66x66 (top/left zero) -> 4 polyphase
    # components [pr, pc, 33, 33] -> row-halves with 1-row halo.
    xf = np.asarray(x, dtype=np.float32)
    xp = np.zeros((B, CIN, 66, 66), dtype=x_np)
    xp[:, :, 1:1 + H, 1:1 + W] = xf
    xph = xp.reshape(B, CIN, 33, 2, 33, 2).transpose(0, 1, 3, 5, 2, 4)
    # xph: [B, CIN, pr, pc, 33, 33]
    halves = np.stack([xph[..., 0:PR, :], xph[..., 33 - PR:33, :]], axis=2)
    # halves: [B, CIN, half, pr, pc, PR, PC]
    # One 17x32 row-major array per (phase, col-shift) combo — every tap's
    # rhs window is then a contiguous 512 block in SBUF.
    arrs = [halves[:, :, :, phase // 2, phase % 2, :, c0:c0 + AW]
            for (phase, c0) in COMBOS]
    xarr = np.stack(arrs, axis=3)  # [B, CIN, half, 6, PR, AW]
    xs = np.ascontiguousarray(
        xarr.reshape(NCORES, BL, KC, 128, NT, XHALF).transpose(
            0, 1, 2, 4, 3, 5))
    # weight: [COUT, CIN, 3, 3] -> [mc, p(cin%128), kc, tap, m(cout%128)]
    wh = np.asarray(weight, dtype=np.float32).transpose(2, 3, 1, 0)  # ky,kx,cin,cout
    wh = wh.reshape(9, KC, 128, MC, 128).transpose(3, 2, 1, 0, 4)
    wh = np.ascontiguousarray(wh.reshape(MC, 128, 9 * KC * 128)).astype(w_np)
    return [{"x": xs[c], "w": wh} for c in range(NCORES)]


def run(x, weight, mm_dtype="f8", nq=16, **spmd_kwargs):
    nc = _get(mm_dtype, nq)
    in_maps = _prep_inputs(x, weight, mm_dtype)
    res = run_bass_kernel_spmd(nc, in_maps, core_ids=list(range(NCORES)),
                               **spmd_kwargs)
    out = np.empty((B, COUT, OH, OW), dtype=np.float32)
    for c in range(NCORES):
        out[c * BL:(c + 1) * BL] = res.results[c]["y"].reshape(
            BL, COUT, OH, OW).astype(np.float32)
    return out, res


def kernel(x, weight):
    out, _ = run(x, weight)
    return out


# revision 34
# speedup vs baseline: 1.0184x; 1.0184x over previous
"""Trainium2 Bass kernel for CropConv (stride-2 3x3 conv, B=32 CIN=COUT=256,
H=W=64 -> 32x32; the crop mask is provably all-ones so output == conv output).

Strategy: data-parallel over batch across 8 NeuronCores (4 images/core).
Host pads each image to 66x66 and splits it into 4 polyphase components
(row/col parity), so each conv tap's moving operand is a fully contiguous
window. Per core the conv is 18 accumulated matmuls per 512-position output
tile: 9 taps x 2 cin-128-chunks, contracting cin on the PE partition dim.
PSUM accumulates fp32.

Precision split: weights are fp16 (11-bit mantissa), x ships as fp8 e3m4
(4-bit mantissa) — the PE upconverts both operands to fp22, so mixed-dtype
matmuls are native. e3m4 x-quantization alone gives 1.33e-2 max-rel error
on the reference inputs (gate 2e-2, deterministic seed); halving x DMA
bytes relaxes the HBM-delivery-bound startup ramp. Output is stored fp16
(|out| <= ~8, quantization ~5e-4 relative) and upcast on host.

Rejected faster paths (measured/derived this session):
- fp8 e4m3 DoubleRow (2x PE rate): needs BOTH operands e4m3; any product
  with an e4m3 factor costs >=2.9e-2 max-rel alone — over the gate. Hi/lo
  pair-splitting restores precision but doubles matmul count (= fp16).
- Winograd F(4,2)/F(2,2) on the polyphase components (1.4-1.8x fewer PE
  cols): the inverse transform needs ~5-12 elementwise ops/output on
  DVE/ACT at ~170ns+1ns/col per instruction — costs far more than the PE
  time saved. PSUM cannot be combined by the PE (write-only) or by DMA
  (no scaled/subtractive accumulate).
- N=1024 moving operands: matmul output is capped at one PSUM bank
  (512 fp32 cols); walrus ISA check s3d3_mm_num_elements rejects it.
- Strassen at the [256,2304]x[2304,1024] level: 7/8 matmuls but the 7
  B-side combos either re-quantize sums in fp8 (error ~2-3e-2) or ship
  1.75x the x bytes in fp16 (delivery-bound).
- Stripping the Bass-init all-engine barrier from main: its drains are
  load-bearing for the runtime handoff (NRT_EXEC_UNIT_UNRECOVERABLE).
- Warm-ups emitted in main pre-context: DMA triggers don't move (7.16us
  is the trigger engines' own preamble floor) and the PE's late context
  entry adds a ~2us delivery stall via the scheduler's clock sems.
- Splitting the FINAL y DMA across sync+scalar parallel triggers: traces
  look right but the device enters a persistent ~15us-slower mode
  (78.6 -> 93.3us, confirmed by immediate differential revert).
- Extending the gpsimd trigger window past the first 12 pieces, reduced
  DMA queue counts, fewer warm-ups, mc-outer group order, whole-chunk
  (1-piece) steady-state x DMAs: all measured worse.

Timeline notes (from ntff traces): the graded exec window runs from the
runtime preamble (~6.1us untraced: engine rendezvous + TENSOR_LOAD) to
the end of the runtime's fixed ~8us semaphore-zeroing teardown; both are
NEFF-runtime costs, invariant to kernel structure. Matmul operands at
fp16/fp8 stream 1 col/cycle (216ns per 512-col matmul, gap-free). Eleven
warm-up matmuls on an UNINITIALIZED raw SBUF scratch (no memset
dependency; NaNs are constant-time in the fp22 PE pipeline and the
warm-up PSUM is never read) start PE activity at context entry, so the
HAM governor un-throttles the PE to 2.4GHz (~4.2us activity window)
about when the first real operands (w0 + image-0 chunks, ~2.0MB) finish
delivery. Fewer warm-ups measured WORSE: real matmuls hit delivery
stalls inside the HAM window and push un-throttle out by ~5us. The final
group is split half/quarter/quarter so earlier drains overlap the last
matmuls and the closing cast->HWDGE-DMA chain handles only 128 columns;
gpsimd (SWDGE) never appears near the end, keeping its dge-drain off the
critical path.
"""

import numpy as np
import ml_dtypes

import concourse.bacc as bacc
import concourse.mybir as mybir
import concourse.tile as tile
from concourse.bass_utils import run_bass_kernel_spmd

B, CIN, COUT, H, W = 32, 256, 256, 64, 64
OH, OW = 32, 32
NCORES = 8
BL = B // NCORES          # images per core
KC = CIN // 128           # cin chunks
MC = COUT // 128          # cout chunks
NT = 2                    # output row-halves per image (16 rows x 32 cols = 512)
RT = OH // NT             # out rows per tile
PR = 17                   # phase rows per half (16 + 1 halo)
PC = 33                   # phase cols
AW = 32                   # stored array width: 32-wide rows make every
                          # tap's rhs window a CONTIGUOUS 512 block
ASZ = PR * AW             # one (phase, col-shift) array = 544 elems
# (phase, c0) arrays stored per x half-tile. Only kx=2 taps read at col
# offset 1, so just phases 0 and 2 need the shifted duplicate.
COMBOS = [(0, 0), (1, 0), (2, 0), (3, 0), (0, 1), (2, 1)]
AIDX = {pc0: i for i, pc0 in enumerate(COMBOS)}
XHALF = len(COMBOS) * ASZ

# taps ordered c0=0 first then by phase — matches the flat array
# order the DMA pieces arrive in.
TAPS_C0 = sorted([(ky, kx) for ky in range(3) for kx in range(3)],
                 key=lambda t: (t[1] // 2, (t[0] % 2) * 2 + (t[1] % 2)))

_CACHE = {}


def _dtypes(mm_dtype):
    """(w_dtype, x_dtype) pairs by mode."""
    return {
        "f8": (mybir.dt.float16, mybir.dt.float8e3),
        "f16": (mybir.dt.float16, mybir.dt.float16),
        "bf16": (mybir.dt.bfloat16, mybir.dt.bfloat16),
    }[mm_dtype]


def _np_dtypes(mm_dtype):
    return {
        "f8": (np.float16, ml_dtypes.float8_e3m4),
        "f16": (np.float16, np.float16),
        "bf16": (ml_dtypes.bfloat16, ml_dtypes.bfloat16),
    }[mm_dtype]


def _build(mm_dtype="f8", nq=16):
    w_dt, x_dt = _dtypes(mm_dtype)
    nc = bacc.Bacc("TRN2", target_bir_lowering=False, debug=False,
                   num_devices=NCORES)
    if nq != 16:
        for q in nc.m.queues:
            q.num_queues = nq
    x = nc.dram_tensor("x", [BL, KC, NT, 128, XHALF], x_dt, kind="ExternalInput")
    w = nc.dram_tensor("w", [MC, 128, 9 * KC * 128], w_dt, kind="ExternalInput")
    # fp16 output: |out| <= ~8 so fp16 quantization adds only ~5e-4
    # relative error; halves output DMA bytes (host upcasts to fp32).
    y = nc.dram_tensor("y", [BL, MC, 128, OH * OW], mybir.dt.float16,
                       kind="ExternalOutput")

    n_warm = 11
    # Raw (non-pool) SBUF scratch for PE warm-up reads: bypasses the Tile
    # framework's written-before-read tracking so no memset is needed.
    warm_sb = nc.alloc_sbuf_tensor("warm_scratch", [128, 128 + RT * OW], w_dt)

    with tile.TileContext(nc) as tc:
        with (
            tc.tile_pool(name="wpool", bufs=MC) as wpool,
            tc.tile_pool(name="xpool", bufs=BL * KC * NT) as xpool,
            tc.tile_pool(name="opool", bufs=12) as opool,
            tc.tile_pool(name="spool", bufs=1) as spool,
            tc.tile_pool(name="psum", bufs=8, space="PSUM") as psum_pool,
        ):
            # PE warm-up: dummy matmuls keep the PE busy through the HAM
            # activity window while the first input DMAs are in flight, so
            # real matmuls start at 2.4 GHz. The scratch tile is read
            # UNINITIALIZED (garbage, possibly NaN): the fp22 PE pipeline
            # is constant-time on NaNs and the PSUM result is never read;
            # skipping the memset starts the HAM ramp ~0.3us earlier.
            if n_warm:
                wps = psum_pool.tile([128, RT * OW], mybir.dt.float32,
                                     name="warm_ps", tag="ps")
                for _ in range(n_warm):
                    nc.tensor.matmul(wps[:], warm_sb.ap()[:, :128],
                                     warm_sb.ap()[:, 128:], start=True,
                                     stop=True)

            # Input DMAs, finest first: the first matmul group gates only on
            # w[0] and image 0's first chunk, so those are split into small
            # pieces issued on independent trigger engines/queues.
            w_sb = {}
            x_sb = {}
            # gpsimd's SWDGE adds a ~3us dge-drain at kernel end, so it only
            # carries the first few critical pieces where a third parallel
            # queue shortens the startup; everything later uses the two
            # HWDGE queues (sync + scalar).
            trig3 = [nc.sync, nc.scalar, nc.gpsimd]
            trig2 = [nc.sync, nc.scalar]
            n_trig = 0

            def next_eng():
                nonlocal n_trig
                trig = trig3 if n_trig < 12 else trig2
                e = trig[n_trig % len(trig)]
                n_trig += 1
                return e

            w_sb[0] = wpool.tile([128, 9 * KC * 128], w_dt, tag="wsb", name="wsb0")
            w_sb[1] = wpool.tile([128, 9 * KC * 128], w_dt, tag="wsb", name="wsb1")

            # DMAs in PE-consumption order, small pieces round-robined over
            # the three trigger queues so each piece arrives just in time.
            for b in range(BL):
                for nt in range(NT):
                    for kc in range(KC):
                        x_sb[(b, kc, nt)] = xpool.tile(
                            [128, XHALF], x_dt,
                            tag="ximg", name=f"x_{b}_{kc}_{nt}")

            def w_piece(mc, i, n):
                step = 9 * KC * 128 // n
                sl = slice(i * step, (i + 1) * step)
                next_eng().dma_start(w_sb[mc][:, sl], w.ap()[mc][:, sl])

            def x_piece(b, kc, nt, i, n):
                step = XHALF // n
                sl = slice(i * step, (i + 1) * step)
                next_eng().dma_start(x_sb[(b, kc, nt)][:, sl],
                                     x.ap()[b, kc, nt, :, sl])

            w_piece(0, 0, 4)
            x_piece(0, 0, 0, 0, 4)
            x_piece(0, 0, 0, 1, 4)
            w_piece(0, 1, 4)
            x_piece(0, 0, 0, 2, 4)
            x_piece(0, 0, 0, 3, 4)
            x_piece(0, 1, 0, 0, 4)
            w_piece(0, 2, 4)
            x_piece(0, 1, 0, 1, 4)
            w_piece(0, 3, 4)
            x_piece(0, 1, 0, 2, 4)
            x_piece(0, 1, 0, 3, 4)
            for i in range(4):
                w_piece(1, i, 4)
            for kc in range(KC):
                for i in range(2):
                    x_piece(0, kc, 1, i, 2)
            for b in range(1, BL):
                for nt in range(NT):
                    for kc in range(KC):
                        for i in range(2):
                            x_piece(b, kc, nt, i, 2)

            out_trig = [nc.sync, nc.scalar]
            n_out = 0

            groups = [(mc, b, nt)
                      for b in range(BL)
                      for nt in range(NT)
                      for mc in range(MC)]
            def do_group(b, nt, mc, rh, n_rh, ps_cols, last_tile):
                """One accumulation group over output rows
                [rh*RT/n_rh, (rh+1)*RT/n_rh) of the (b, nt) half."""
                nonlocal n_out
                rt = RT // n_rh
                ps = psum_pool.tile([128, ps_cols], mybir.dt.float32,
                                    name=f"ps_{b}_{mc}_{nt}_{rh}", tag="ps")
                n_mm = KC * len(TAPS_C0)
                i_mm = 0
                for kc in range(KC):
                    xt = x_sb[(b, kc, nt)]
                    for (ky, kx) in TAPS_C0:
                        phase = (ky % 2) * 2 + (kx % 2)
                        r0 = ky // 2 + rh * rt
                        c0 = kx // 2
                        base = AIDX[(phase, c0)] * ASZ + r0 * AW
                        rhs = xt[:, base:base + rt * OW]
                        lhsT = w_sb[mc][:, (kc * 9 + ky * 3 + kx)
                                        * 128:][:, :128]
                        nc.tensor.matmul(
                            ps[:], lhsT, rhs,
                            start=(i_mm == 0), stop=(i_mm == n_mm - 1),
                        )
                        i_mm += 1
                base = nt * 512 + rh * rt * OW
                n_ch = 2 if n_rh == 1 else 1
                chunk = rt * OW // n_ch
                for h in range(n_ch):
                    ot = opool.tile([128, chunk], mybir.dt.float16,
                                    tag="ostage")
                    if last_tile:
                        # Tail latency: copy on an idle engine, DMA on the
                        # idle HWDGE queues (gpsimd SWDGE would add a
                        # dge-drain right at kernel end).
                        ceng = nc.vector if h == 0 else nc.scalar
                        deng = nc.sync if h == 0 else nc.scalar
                    else:
                        ceng = nc.vector
                        deng = out_trig[n_out % len(out_trig)]
                    n_out += 1  # round-robin across trigger engines
                    if ceng is nc.scalar:
                        ceng.copy(ot[:], ps[:, h * chunk:(h + 1) * chunk])
                    else:
                        ceng.tensor_copy(
                            ot[:], ps[:, h * chunk:(h + 1) * chunk])
                    deng.dma_start(
                        y.ap()[b, mc, :,
                               base + h * chunk:base + (h + 1) * chunk],
                        ot[:],
                    )

            for i_group, (mc, b, nt) in enumerate(groups):
                if i_group < len(groups) - 1:
                    do_group(b, nt, mc, 0, 1, RT * OW, False)
                else:
                    # Final group: a half then two quarter-height
                    # accumulations so earlier drains overlap the last
                    # matmuls and the closing cast->DMA chain handles
                    # only 128 columns.
                    do_group(b, nt, mc, 0, 2, RT * OW // 2, False)
                    do_group(b, nt, mc, 2, 4, RT * OW // 4, False)
                    do_group(b, nt, mc, 3, 4, RT * OW // 4, True)
    nc.compile()
    # Bass init unconditionally emits four const-AP memsets (fp32 0/1,
    # bf16 1, uint8 127) that nothing in this kernel reads — dead code
    # that also happens to run ~1.1us before the first DMA trigger.
    for blk in nc.m.functions[0].blocks:
        blk.instructions[:] = [
            i for i in blk.instructions
            if not ("Memset" in type(i).__name__ and "const-" in i.concise())
        ]
    return nc


def _get(mm_dtype="f8", nq=16):
    key = (mm_dtype, nq)
    if key not in _CACHE:
        _CACHE[key] = _build(mm_dtype, nq)
    return _CACHE[key]


def _prep_inputs(x, weight, mm_dtype="f8"):
    w_np, x_np = _np_dtypes(mm_dtype)
    # x: [B, CIN, H, W] -> pad to 66x66 (top/left zero) -> 4 polyphase
    # components [pr, pc, 33, 33] -> row-halves with 1-row halo.
    xf = np.asarray(x, dtype=np.float32)
    xp = np.zeros((B, CIN, 66, 66), dtype=x_np)
    xp[:, :, 1:1 + H, 1:1 + W] = xf
    xph = xp.reshape(B, CIN, 33, 2, 33, 2).transpose(0, 1, 3, 5, 2, 4)
    # xph: [B, CIN, pr, pc, 33, 33]
    halves = np.stack([xph[..., 0:PR, :], xph[..., 33 - PR:33, :]], axis=2)
    # halves: [B, CIN, half, pr, pc, PR, PC]
    # One 17x32 row-major array per (phase, col-shift) combo — every tap's
    # rhs window is then a contiguous 512 block in SBUF.
    arrs = [halves[:, :, :, phase // 2, phase % 2, :, c0:c0 + AW]
            for (phase, c0) in COMBOS]
    xarr = np.stack(arrs, axis=3)  # [B, CIN, half, 6, PR, AW]
    xs = np.ascontiguousarray(
        xarr.reshape(NCORES, BL, KC, 128, NT, XHALF).transpose(
            0, 1, 2, 4, 3, 5))
    # weight: [COUT, CIN, 3, 3] -> [mc, p(cin%128), kc, tap, m(cout%128)]
    wh = np.asarray(weight, dtype=np.float32).transpose(2, 3, 1, 0)  # ky,kx,cin,cout
    wh = wh.reshape(9, KC, 128, MC, 128).transpose(3, 2, 1, 0, 4)
    wh = np.ascontiguousarray(wh.reshape(MC, 128, 9 * KC * 128)).astype(w_np)
    return [{"x": xs[c], "w": wh} for c in range(NCORES)]


def run(x, weight, mm_dtype="f8", nq=16, **spmd_kwargs):
    nc = _get(mm_dtype, nq)
    in_maps = _prep_inputs(x, weight, mm_dtype)
    res = run_bass_kernel_spmd(nc, in_maps, core_ids=list(range(NCORES)),
                               **spmd_kwargs)
    out = np.empty((B, COUT, OH, OW), dtype=np.float32)
    for c in range(NCORES):
        out[c * BL:(c + 1) * BL] = res.results[c]["y"].reshape(
            BL, COUT, OH, OW).astype(np.float32)
    return out, res


def kernel(x, weight):
    out, _ = run(x, weight)
    return out
